# revision 13
# baseline (speedup 1.0000x reference)
"""NT-Xent loss on 8 Trainium2 cores (v3b: fp8 DoubleRow + diag kill + accum split).

Math: with row-normalized views zjn, zin and r = [zjn; zin],
S = r@r.T / T, pos_i = (zjn_i . zin_i)/T, the kept logits for row i are
the same-view off-diagonal entries plus pos_i.  All cosine logits are
<= 1/T = 10, so with the fixed shift 10:

  lse_i  = 10 + ln( rowsum_i + epos_i )
  loss   = mean(lse_i - pos_i)

where rowsum_i = sum_{j != i} exp(S_same[i,j] - 10) and
epos_i = exp(pos_i - 10).

Device (SPMD, cores 0-3 view zj, cores 4-7 view zi; each owns a
1024-row slab): rows prescaled by 16, quantized to fp8e4m3.  Per-core
anT columns are rotated by -slab*1024 so each core's own rows occupy
columns [0,1024) -- its Gram diagonal then sits at fixed positions
(tile t, cols t*128..t*128+128, entry [p, t*128+p]) identical across
cores.  An extra identity matmul adds -480*I there before the exp, so
exp(<= -16) ~ 0 removes the diagonal on device.
G = qnT.T @ anT via DoubleRow matmuls, ACT exp(G*(10/256) - 10).
Row-sum split: h=0 half reduced on DVE (hidden behind ACT), h=1 half
summed by the ACT accumulator itself (accum_out).  Inputs spread over
5 DMA queues; a warmup matmul removes the cold-PE penalty.
Host does the O(N*D) rest (normalize, pos, log, mean).
"""

import numpy as np
import ml_dtypes

N = 4096
D = 256
TEMP = 0.1
NCORES = 8
RPC = 2 * N // NCORES          # 1024 rows per core
IT = RPC // 128                # 8 i-tiles of 128 rows
HALF = 2048                    # j-chunk per PSUM buffer / ACT op
NH = N // HALF                 # 2 halves of the 4096-wide Gram row
NCH = HALF // 512              # 4 column chunks per half
SC = 16.0                      # fp8 prescale (power of 2, exact)
ASCALE = (1.0 / TEMP) / (SC * SC)   # 10/256 applied in ACT
DIAGK = 240.0                  # fp8e4m3 max; with idt=2*I the diag gets -480

_CACHE = {}


def _build_program():
    if "nc" in _CACHE:
        return _CACHE["nc"]

    import concourse.bass as bass
    import concourse.tile as tile
    from concourse import bacc, mybir

    F8 = mybir.dt.float8e4
    BF16 = mybir.dt.bfloat16
    F32 = mybir.dt.float32

    nc = bacc.Bacc(
        "TRN2", target_bir_lowering=False, debug=False, num_devices=NCORES
    )

    # anT[h][c][p][k][col] = a8rot[h*2048 + c*512 + col, k*128 + p]
    anT_d = nc.dram_tensor("anT", [NH, NCH, 128, 2, 512], F8, kind="ExternalInput")
    # qnT[p][k][r] = q8slab[r, k*128 + p]
    qnT_d = nc.dram_tensor("qnT", [128, 2, RPC], F8, kind="ExternalInput")
    idt_d = nc.dram_tensor("idt", [128, 128], F8, kind="ExternalInput")
    ngid_d = nc.dram_tensor("ngid", [128, 128], F8, kind="ExternalInput")
    acc_d = nc.dram_tensor("acc", [128, IT, NH], F32, kind="ExternalOutput")

    with tile.TileContext(nc) as tc:
        with (
            tc.tile_pool(name="weights", bufs=1) as wpool,
            tc.tile_pool(name="scratch", bufs=2) as spool,
            tc.tile_pool(name="psum", bufs=2, space="PSUM") as ppool,
        ):
            qnT = wpool.tile([128, 2, RPC], F8)
            an = [
                [wpool.tile([128, 2, 512], F8, name=f"an{h}_{c}") for c in range(NCH)]
                for h in range(NH)
            ]
            idt = wpool.tile([128, 128], F8)
            ngid = wpool.tile([128, 128], F8)
            # spread input DMAs over 3 queues; t0-critical pieces first
            nc.sync.dma_start(out=qnT[:], in_=qnT_d[:])
            nc.sync.dma_start(out=an[0][0][:], in_=anT_d[0, 0])
            nc.sync.dma_start(out=an[0][2][:], in_=anT_d[0, 2])
            nc.sync.dma_start(out=an[0][3][:], in_=anT_d[0, 3])
            nc.gpsimd.dma_start(out=idt[:], in_=idt_d[:])
            nc.gpsimd.dma_start(out=ngid[:], in_=ngid_d[:])
            nc.gpsimd.dma_start(out=an[0][1][:], in_=anT_d[0, 1])
            for c in range(NCH):
                nc.scalar.dma_start(out=an[1][c][:], in_=anT_d[1, c])

            acc = wpool.tile([128, IT, NH], F32)
            scrap = wpool.tile([128, HALF], BF16)
            bias = wpool.tile([128, 1], F32)
            nc.vector.memset(bias[:], -1.0 / TEMP)

            for t in range(IT):
                # ---- h = 0 half (holds the diagonal): ACT -> sc, DVE reduce
                ps = ppool.tile([128, HALF], F32)
                cd = t // 4
                order = [cd] + [c for c in range(NCH) if c != cd]
                for c in order:
                    nc.tensor.matmul(
                        ps[:, c * 512:(c + 1) * 512],
                        qnT[:, :, t * 128:(t + 1) * 128],
                        an[0][c][:],
                        start=True,
                        stop=(c != cd),
                        perf_mode=mybir.MatmulPerfMode.DoubleRow,
                        skip_group_check=(c == cd),
                    )
                    if c == cd:
                        nc.tensor.matmul(
                            ps[:, t * 128:(t + 1) * 128],
                            idt[:],
                            ngid[:],
                            start=False,
                            stop=True,
                            skip_group_check=True,
                        )
                sc = spool.tile([128, HALF], BF16)
                if t == 0:
                    # split first ACT so the scalar chain starts ~2 matmuls earlier
                    for a in range(2):
                        nc.scalar.activation(
                            sc[:, a * 1024:(a + 1) * 1024],
                            ps[:, a * 1024:(a + 1) * 1024],
                            mybir.ActivationFunctionType.Exp,
                            bias=bias[:],
                            scale=ASCALE,
                        )
                else:
                    nc.scalar.activation(
                        sc[:],
                        ps[:],
                        mybir.ActivationFunctionType.Exp,
                        bias=bias[:],
                        scale=ASCALE,
                    )
                nc.vector.tensor_reduce(
                    acc[:, t, 0:1],
                    sc[:],
                    axis=mybir.AxisListType.X,
                    op=mybir.AluOpType.add,
                )

                # ---- h = 1 half: ACT accumulator does the row sum
                ps = ppool.tile([128, HALF], F32)
                for c in range(NCH):
                    nc.tensor.matmul(
                        ps[:, c * 512:(c + 1) * 512],
                        qnT[:, :, t * 128:(t + 1) * 128],
                        an[1][c][:],
                        start=True,
                        stop=True,
                        perf_mode=mybir.MatmulPerfMode.DoubleRow,
                    )
                nc.scalar.activation(
                    scrap[:],
                    ps[:],
                    mybir.ActivationFunctionType.Exp,
                    bias=bias[:],
                    scale=ASCALE,
                    accum_out=acc[:, t, 1:2],
                )

            nc.sync.dma_start(out=acc_d[:], in_=acc[:])

    nc.compile()
    _CACHE["nc"] = nc
    return nc


def _prep_inputs(z_i, z_j):
    f8 = ml_dtypes.float8_e4m3
    zin = z_i / np.sqrt(np.sum(z_i * z_i, axis=1, keepdims=True))
    zjn = z_j / np.sqrt(np.sum(z_j * z_j, axis=1, keepdims=True))
    posn = np.sum(zin * zjn, axis=1, dtype=np.float64) / TEMP      # [4096]

    q8 = [(SC * zjn).astype(f8), (SC * zin).astype(f8)]
    ident = (2.0 * np.eye(128)).astype(f8)
    negid = (-DIAGK * np.eye(128)).astype(f8)

    in_maps = []
    for c in range(NCORES):
        v = c // (NCORES // 2)
        s = c % (NCORES // 2)
        b = q8[v]
        brot = np.roll(b, -s * RPC, axis=0)            # own slab -> cols [0,1024)
        bT = brot.T                                    # [256, 4096]
        anT = np.ascontiguousarray(
            bT.reshape(2, 128, NH, NCH, 512).transpose(2, 3, 1, 0, 4)
        )
        slab = b[s * RPC:(s + 1) * RPC]
        qnT = np.ascontiguousarray(slab.T.reshape(2, 128, RPC).transpose(1, 0, 2))
        in_maps.append({"anT": anT, "qnT": qnT, "idt": ident, "ngid": negid})
    return in_maps, posn


def kernel(z_i, z_j):
    z_i = np.asarray(z_i, dtype=np.float32)
    z_j = np.asarray(z_j, dtype=np.float32)

    from concourse.bass_utils import run_bass_kernel_spmd

    nc = _build_program()
    in_maps, posn = _prep_inputs(z_i, z_j)

    res = run_bass_kernel_spmd(nc, in_maps, list(range(NCORES)))
    _CACHE["last_results"] = res

    rowsum = np.empty(2 * N, dtype=np.float64)
    for c in range(NCORES):
        a = res.results[c]["acc"].astype(np.float64)   # [128, IT, NH]
        slab = a.sum(axis=2).T.reshape(-1)             # [1024], row t*128+p
        rowsum[c * RPC:(c + 1) * RPC] = slab

    posn_g = np.concatenate([posn, posn])
    epos_g = np.exp(posn_g - 1.0 / TEMP)

    lse = 1.0 / TEMP + np.log(rowsum + epos_g)
    loss = np.mean(lse - posn_g)
    return np.array(loss, dtype=np.float32)


# revision 14
# speedup vs baseline: 1.2248x; 1.2248x over previous
"""NT-Xent loss on 8 Trainium2 cores (v3b: fp8 DoubleRow + diag kill + accum split).

Math: with row-normalized views zjn, zin and r = [zjn; zin],
S = r@r.T / T, pos_i = (zjn_i . zin_i)/T, the kept logits for row i are
the same-view off-diagonal entries plus pos_i.  All cosine logits are
<= 1/T = 10, so with the fixed shift 10:

  lse_i  = 10 + ln( rowsum_i + epos_i )
  loss   = mean(lse_i - pos_i)

where rowsum_i = sum_{j != i} exp(S_same[i,j] - 10) and
epos_i = exp(pos_i - 10).

Device (SPMD, cores 0-3 view zj, cores 4-7 view zi; each owns a
1024-row slab): rows prescaled by 16, quantized to fp8e4m3.  Per-core
anT columns are rotated by -slab*1024 so each core's own rows occupy
columns [0,1024) -- its Gram diagonal then sits at fixed positions
(tile t, cols t*128..t*128+128, entry [p, t*128+p]) identical across
cores.  An extra identity matmul adds -480*I there before the exp, so
exp(<= -16) ~ 0 removes the diagonal on device.
G = qnT.T @ anT via DoubleRow matmuls, ACT exp(G*(10/256) - 10).
Row-sum split: h=0 half reduced on DVE (hidden behind ACT), h=1 half
summed by the ACT accumulator itself (accum_out).  Inputs spread over
5 DMA queues; a warmup matmul removes the cold-PE penalty.
Host does the O(N*D) rest (normalize, pos, log, mean).
"""

import numpy as np
import ml_dtypes

N = 4096
D = 256
TEMP = 0.1
NCORES = 8
RPC = 2 * N // NCORES          # 1024 rows per core
IT = RPC // 128                # 8 i-tiles of 128 rows
HALF = 2048                    # j-chunk per PSUM buffer / ACT op
NH = N // HALF                 # 2 halves of the 4096-wide Gram row
NCH = HALF // 512              # 4 column chunks per half
SC = 16.0                      # fp8 prescale (power of 2, exact)
ASCALE = (1.0 / TEMP) / (SC * SC)   # 10/256 applied in ACT
DIAGK = 240.0                  # fp8e4m3 max; with idt=2*I the diag gets -480

_CACHE = {}


def _build_program():
    if "nc" in _CACHE:
        return _CACHE["nc"]

    import concourse.bass as bass
    import concourse.tile as tile
    from concourse import bacc, mybir

    F8 = mybir.dt.float8e4
    BF16 = mybir.dt.bfloat16
    F32 = mybir.dt.float32

    nc = bacc.Bacc(
        "TRN2", target_bir_lowering=False, debug=False, num_devices=NCORES
    )

    # anT[h][c][p][k][col] = a8rot[h*2048 + c*512 + col, k*128 + p]
    anT_d = nc.dram_tensor("anT", [NH, NCH, 128, 2, 512], F8, kind="ExternalInput")
    # qnT[p][k][r] = q8slab[r, k*128 + p]
    qnT_d = nc.dram_tensor("qnT", [128, 2, RPC], F8, kind="ExternalInput")
    idt_d = nc.dram_tensor("idt", [128, 128], F8, kind="ExternalInput")
    ngid_d = nc.dram_tensor("ngid", [128, 128], F8, kind="ExternalInput")
    acc_d = nc.dram_tensor("acc", [128, IT, NH], F32, kind="ExternalOutput")

    with tile.TileContext(nc) as tc:
        with (
            tc.tile_pool(name="weights", bufs=1) as wpool,
            tc.tile_pool(name="scratch", bufs=2) as spool,
            tc.tile_pool(name="psum", bufs=2, space="PSUM") as ppool,
        ):
            qnT = wpool.tile([128, 2, RPC], F8)
            an = [
                [wpool.tile([128, 2, 512], F8, name=f"an{h}_{c}") for c in range(NCH)]
                for h in range(NH)
            ]
            idt = wpool.tile([128, 128], F8)
            ngid = wpool.tile([128, 128], F8)
            # spread input DMAs over 3 queues; t0-critical pieces first
            nc.sync.dma_start(out=qnT[:], in_=qnT_d[:])
            nc.sync.dma_start(out=an[0][0][:], in_=anT_d[0, 0])
            nc.sync.dma_start(out=an[0][2][:], in_=anT_d[0, 2])
            nc.sync.dma_start(out=an[0][3][:], in_=anT_d[0, 3])
            nc.gpsimd.dma_start(out=idt[:], in_=idt_d[:])
            nc.gpsimd.dma_start(out=ngid[:], in_=ngid_d[:])
            nc.gpsimd.dma_start(out=an[0][1][:], in_=anT_d[0, 1])
            for c in range(NCH):
                nc.scalar.dma_start(out=an[1][c][:], in_=anT_d[1, c])

            acc = wpool.tile([128, IT, NH], F32)
            scrap = wpool.tile([128, HALF], BF16)
            bias = wpool.tile([128, 1], F32)
            nc.vector.memset(bias[:], -1.0 / TEMP)

            for t in range(IT):
                # ---- h = 0 half (holds the diagonal): ACT -> sc, DVE reduce
                ps = ppool.tile([128, HALF], F32)
                cd = t // 4
                order = [cd] + [c for c in range(NCH) if c != cd]
                for c in order:
                    nc.tensor.matmul(
                        ps[:, c * 512:(c + 1) * 512],
                        qnT[:, :, t * 128:(t + 1) * 128],
                        an[0][c][:],
                        start=True,
                        stop=(c != cd),
                        perf_mode=mybir.MatmulPerfMode.DoubleRow,
                        skip_group_check=(c == cd),
                    )
                    if c == cd:
                        nc.tensor.matmul(
                            ps[:, t * 128:(t + 1) * 128],
                            idt[:],
                            ngid[:],
                            start=False,
                            stop=True,
                            skip_group_check=True,
                        )
                sc = spool.tile([128, HALF], BF16)
                nc.scalar.activation(
                    sc[:],
                    ps[:],
                    mybir.ActivationFunctionType.Exp,
                    bias=bias[:],
                    scale=ASCALE,
                )
                nc.vector.tensor_reduce(
                    acc[:, t, 0:1],
                    sc[:],
                    axis=mybir.AxisListType.X,
                    op=mybir.AluOpType.add,
                )

                # ---- h = 1 half: ACT accumulator does the row sum
                ps = ppool.tile([128, HALF], F32)
                for c in range(NCH):
                    nc.tensor.matmul(
                        ps[:, c * 512:(c + 1) * 512],
                        qnT[:, :, t * 128:(t + 1) * 128],
                        an[1][c][:],
                        start=True,
                        stop=True,
                        perf_mode=mybir.MatmulPerfMode.DoubleRow,
                    )
                nc.scalar.activation(
                    scrap[:],
                    ps[:],
                    mybir.ActivationFunctionType.Exp,
                    bias=bias[:],
                    scale=ASCALE,
                    accum_out=acc[:, t, 1:2],
                )

            nc.sync.dma_start(out=acc_d[:], in_=acc[:])

    nc.compile()
    _CACHE["nc"] = nc
    return nc


def _prep_inputs(z_i, z_j):
    f8 = ml_dtypes.float8_e4m3
    zin = z_i / np.sqrt(np.sum(z_i * z_i, axis=1, keepdims=True))
    zjn = z_j / np.sqrt(np.sum(z_j * z_j, axis=1, keepdims=True))
    posn = np.sum(zin * zjn, axis=1, dtype=np.float64) / TEMP      # [4096]

    q8 = [(SC * zjn).astype(f8), (SC * zin).astype(f8)]
    ident = (2.0 * np.eye(128)).astype(f8)
    negid = (-DIAGK * np.eye(128)).astype(f8)

    in_maps = []
    for c in range(NCORES):
        v = c // (NCORES // 2)
        s = c % (NCORES // 2)
        b = q8[v]
        brot = np.roll(b, -s * RPC, axis=0)            # own slab -> cols [0,1024)
        bT = brot.T                                    # [256, 4096]
        anT = np.ascontiguousarray(
            bT.reshape(2, 128, NH, NCH, 512).transpose(2, 3, 1, 0, 4)
        )
        slab = b[s * RPC:(s + 1) * RPC]
        qnT = np.ascontiguousarray(slab.T.reshape(2, 128, RPC).transpose(1, 0, 2))
        in_maps.append({"anT": anT, "qnT": qnT, "idt": ident, "ngid": negid})
    return in_maps, posn


def kernel(z_i, z_j):
    z_i = np.asarray(z_i, dtype=np.float32)
    z_j = np.asarray(z_j, dtype=np.float32)

    from concourse.bass_utils import run_bass_kernel_spmd

    nc = _build_program()
    in_maps, posn = _prep_inputs(z_i, z_j)

    res = run_bass_kernel_spmd(nc, in_maps, list(range(NCORES)))
    _CACHE["last_results"] = res

    rowsum = np.empty(2 * N, dtype=np.float64)
    for c in range(NCORES):
        a = res.results[c]["acc"].astype(np.float64)   # [128, IT, NH]
        slab = a.sum(axis=2).T.reshape(-1)             # [1024], row t*128+p
        rowsum[c * RPC:(c + 1) * RPC] = slab

    posn_g = np.concatenate([posn, posn])
    epos_g = np.exp(posn_g - 1.0 / TEMP)

    lse = 1.0 / TEMP + np.log(rowsum + epos_g)
    loss = np.mean(lse - posn_g)
    return np.array(loss, dtype=np.float32)


# revision 15
# speedup vs baseline: 1.2538x; 1.0237x over previous
"""NT-Xent loss on 8 Trainium2 cores (v3c: fp8 DoubleRow + accum split + host diag).

Math: with row-normalized views zjn, zin and r = [zjn; zin],
S = r@r.T / T, pos_i = (zjn_i . zin_i)/T, the kept logits for row i are
the same-view off-diagonal entries plus pos_i.  All cosine logits are
<= 1/T = 10, so with the fixed shift 10:

  lse_i  = 10 + ln( rowsum_i + epos_i )
  loss   = mean(lse_i - pos_i)

where rowsum_i = sum_{j != i} exp(S_same[i,j] - 10) and
epos_i = exp(pos_i - 10).

Device (SPMD, cores 0-3 view zj, cores 4-7 view zi; each owns a
1024-row slab): rows prescaled by 16, quantized to fp8e4m3.
G = qnT.T @ anT via DoubleRow matmuls, ACT exp(G*(10/256) - 10).
Row-sum split: h=0 half reduced on DVE (hidden behind ACT), h=1 half
summed by the ACT accumulator itself (accum_out).  The Gram diagonal
is included on device and subtracted exactly on the host using the
fp8 row norms (fp32 partial sums make the cancellation benign).
Host does the O(N*D) rest (normalize, pos, log, mean).
"""

import numpy as np
import ml_dtypes

N = 4096
D = 256
TEMP = 0.1
NCORES = 8
RPC = 2 * N // NCORES          # 1024 rows per core
IT = RPC // 128                # 8 i-tiles of 128 rows
HALF = 2048                    # j-chunk per PSUM buffer / ACT op
NH = N // HALF                 # 2 halves of the 4096-wide Gram row
NCH = HALF // 512              # 4 column chunks per half
SC = 16.0                      # fp8 prescale (power of 2, exact)
ASCALE = (1.0 / TEMP) / (SC * SC)   # 10/256 applied in ACT

_CACHE = {}


def _build_program():
    if "nc" in _CACHE:
        return _CACHE["nc"]

    import concourse.bass as bass
    import concourse.tile as tile
    from concourse import bacc, mybir

    F8 = mybir.dt.float8e4
    BF16 = mybir.dt.bfloat16
    F32 = mybir.dt.float32

    nc = bacc.Bacc(
        "TRN2", target_bir_lowering=False, debug=False, num_devices=NCORES
    )

    # anT[h][c][p][k][col] = a8[h*2048 + c*512 + col, k*128 + p]
    anT_d = nc.dram_tensor("anT", [NH, NCH, 128, 2, 512], F8, kind="ExternalInput")
    # qnT[p][k][r] = q8slab[r, k*128 + p]
    qnT_d = nc.dram_tensor("qnT", [128, 2, RPC], F8, kind="ExternalInput")
    acc_d = nc.dram_tensor("acc", [128, IT, NH], F32, kind="ExternalOutput")

    with tile.TileContext(nc) as tc:
        with (
            tc.tile_pool(name="weights", bufs=1) as wpool,
            tc.tile_pool(name="scratch", bufs=2) as spool,
            tc.tile_pool(name="psum", bufs=2, space="PSUM") as ppool,
        ):
            qnT = wpool.tile([128, 2, RPC], F8)
            an = [
                [wpool.tile([128, 2, 512], F8, name=f"an{h}_{c}") for c in range(NCH)]
                for h in range(NH)
            ]
            # spread input DMAs over 3 queues; t0-critical pieces first
            nc.sync.dma_start(out=qnT[:], in_=qnT_d[:])
            nc.sync.dma_start(out=an[0][0][:], in_=anT_d[0, 0])
            nc.sync.dma_start(out=an[0][2][:], in_=anT_d[0, 2])
            nc.gpsimd.dma_start(out=an[0][1][:], in_=anT_d[0, 1])
            nc.gpsimd.dma_start(out=an[0][3][:], in_=anT_d[0, 3])
            for c in range(NCH):
                nc.scalar.dma_start(out=an[1][c][:], in_=anT_d[1, c])

            acc = wpool.tile([128, IT, NH], F32)
            scrap = wpool.tile([128, HALF], BF16)
            bias = wpool.tile([128, 1], F32)
            nc.vector.memset(bias[:], -1.0 / TEMP)

            for t in range(IT):
                # ---- h = 0 half: ACT -> sc, DVE reduce (hidden behind ACT)
                ps = ppool.tile([128, HALF], F32)
                for c in range(NCH):
                    nc.tensor.matmul(
                        ps[:, c * 512:(c + 1) * 512],
                        qnT[:, :, t * 128:(t + 1) * 128],
                        an[0][c][:],
                        start=True,
                        stop=True,
                        perf_mode=mybir.MatmulPerfMode.DoubleRow,
                    )
                sc = spool.tile([128, HALF], BF16)
                nc.scalar.activation(
                    sc[:],
                    ps[:],
                    mybir.ActivationFunctionType.Exp,
                    bias=bias[:],
                    scale=ASCALE,
                )
                nc.vector.tensor_reduce(
                    acc[:, t, 0:1],
                    sc[:],
                    axis=mybir.AxisListType.X,
                    op=mybir.AluOpType.add,
                )

                # ---- h = 1 half: ACT accumulator does the row sum
                ps = ppool.tile([128, HALF], F32)
                for c in range(NCH):
                    nc.tensor.matmul(
                        ps[:, c * 512:(c + 1) * 512],
                        qnT[:, :, t * 128:(t + 1) * 128],
                        an[1][c][:],
                        start=True,
                        stop=True,
                        perf_mode=mybir.MatmulPerfMode.DoubleRow,
                    )
                nc.scalar.activation(
                    scrap[:],
                    ps[:],
                    mybir.ActivationFunctionType.Exp,
                    bias=bias[:],
                    scale=ASCALE,
                    accum_out=acc[:, t, 1:2],
                )

            nc.sync.dma_start(out=acc_d[:], in_=acc[:])

    nc.compile()
    _CACHE["nc"] = nc
    return nc


def _prep_inputs(z_i, z_j):
    f8 = ml_dtypes.float8_e4m3
    zin = z_i / np.sqrt(np.sum(z_i * z_i, axis=1, keepdims=True))
    zjn = z_j / np.sqrt(np.sum(z_j * z_j, axis=1, keepdims=True))
    posn = np.sum(zin * zjn, axis=1, dtype=np.float64) / TEMP      # [4096]

    q8 = [(SC * zjn).astype(f8), (SC * zin).astype(f8)]
    # exact squared norms of the quantized rows: the device Gram diagonal
    dsq = [np.sum(b.astype(np.float64) ** 2, axis=1) for b in q8]

    in_maps = []
    for c in range(NCORES):
        v = c // (NCORES // 2)
        b = q8[v]
        bT = b.T                                       # [256, 4096]
        anT = np.ascontiguousarray(
            bT.reshape(2, 128, NH, NCH, 512).transpose(2, 3, 1, 0, 4)
        )
        s = c % (NCORES // 2)
        slab = b[s * RPC:(s + 1) * RPC]
        qnT = np.ascontiguousarray(slab.T.reshape(2, 128, RPC).transpose(1, 0, 2))
        in_maps.append({"anT": anT, "qnT": qnT})
    return in_maps, posn, dsq


def kernel(z_i, z_j):
    z_i = np.asarray(z_i, dtype=np.float32)
    z_j = np.asarray(z_j, dtype=np.float32)

    from concourse.bass_utils import run_bass_kernel_spmd

    nc = _build_program()
    in_maps, posn, dsq = _prep_inputs(z_i, z_j)

    res = run_bass_kernel_spmd(nc, in_maps, list(range(NCORES)))
    _CACHE["last_results"] = res

    rowsum = np.empty(2 * N, dtype=np.float64)
    for c in range(NCORES):
        a = res.results[c]["acc"].astype(np.float64)   # [128, IT, NH]
        slab = a.sum(axis=2).T.reshape(-1)             # [1024], row t*128+p
        rowsum[c * RPC:(c + 1) * RPC] = slab

    dsq_g = np.concatenate(dsq)                        # [8192] |q8 row|^2
    rowsum -= np.exp(dsq_g * ASCALE - 1.0 / TEMP)      # exact diagonal removal

    posn_g = np.concatenate([posn, posn])
    epos_g = np.exp(posn_g - 1.0 / TEMP)

    lse = 1.0 / TEMP + np.log(rowsum + epos_g)
    loss = np.mean(lse - posn_g)
    return np.array(loss, dtype=np.float32)


# revision 21
# speedup vs baseline: 1.4343x; 1.1440x over previous
"""NT-Xent loss on 8 Trainium2 cores (v4: cyclic 3-block symmetry, 75% exp work).

Math: with row-normalized views zjn, zin and r = [zjn; zin],
S = r@r.T / T, pos_i = (zjn_i . zin_i)/T, the kept logits for row i are
the same-view off-diagonal entries plus pos_i.  All cosine logits are
<= 1/T = 10, so with the fixed shift 10:

  lse_i  = 10 + ln( rowsum_i + epos_i )
  loss   = mean(lse_i - pos_i)

where rowsum_i = sum_{j != i} exp(S_same[i,j] - 10) and
epos_i = exp(pos_i - 10).

Symmetry: each view's 4096x4096 Gram is split into 4x4 blocks of
1024.  Core (v,s) computes its row-slab against column blocks
{s, s+1, s+2} (cyclic), i.e. 3072 of 4096 columns.  The missing block
(s, s+3) equals block (s+3, s).T, which core (v, s+3) computes as its
"+1" block; its COLUMN sums are that block's row sums.  So each core
also accumulates the column sums of its +1 block (DVE adds into a
[128,1024] fp32 accumulator; host finishes the partition reduction).

Device per tile t (hB first so the accum-ACT ends the chain):
  3 DoubleRow fp8 MMs -> psB[128,1536]; ACT exp -> scB;
  DVE row-reduce scB -> acc[:,t,0]; DVE colacc += scB[:,512:1536];
  3 MMs -> psA; ACT exp with accum_out=acc[:,t,1] (out to scrap).
Two garbage DoubleRow warm-up MMs run first to take the PE out of its
cold state.  Host: rowsum(v,s) = slabsum + colsum from core (v,s-1)
- exact fp8 diagonal; then the O(N*D) rest (normalize, pos, log, mean).
"""

import numpy as np
import ml_dtypes

N = 4096
D = 256
TEMP = 0.1
NCORES = 8
RPC = 2 * N // NCORES          # 1024 rows per core
IT = RPC // 128                # 8 i-tiles of 128 rows
W = 3 * RPC                    # 3072 columns per core (3 cyclic blocks)
HALFW = W // 2                 # 1536 cols per PSUM buffer / ACT op
NCH = HALFW // 512             # 3 column chunks per half
SC = 16.0                      # fp8 prescale (power of 2, exact)
ASCALE = (1.0 / TEMP) / (SC * SC)   # 10/256 applied in ACT

_CACHE = {}


def _build_program():
    if "nc" in _CACHE:
        return _CACHE["nc"]

    import concourse.bass as bass
    import concourse.tile as tile
    from concourse import bacc, mybir

    F8 = mybir.dt.float8e4
    BF16 = mybir.dt.bfloat16
    F32 = mybir.dt.float32

    nc = bacc.Bacc(
        "TRN2", target_bir_lowering=False, debug=False, num_devices=NCORES
    )

    # anT[h][c][p][k][col] = cols[h*1536 + c*512 + col, k*128 + p]
    anT_d = nc.dram_tensor("anT", [2, NCH, 128, 2, 512], F8, kind="ExternalInput")
    # qnT[p][k][r] = q8slab[r, k*128 + p]
    qnT_d = nc.dram_tensor("qnT", [128, 2, RPC], F8, kind="ExternalInput")
    acc_d = nc.dram_tensor("acc", [128, IT, 2], F32, kind="ExternalOutput")
    col_d = nc.dram_tensor("colacc", [128, RPC], F32, kind="ExternalOutput")

    with tile.TileContext(nc) as tc:
        with (
            tc.tile_pool(name="weights", bufs=1) as wpool,
            tc.tile_pool(name="scratch", bufs=2) as spool,
            tc.tile_pool(name="psum", bufs=2, space="PSUM") as ppool,
            tc.tile_pool(name="pwarm", bufs=1, space="PSUM") as wppool,
        ):
            qnT = wpool.tile([128, 2, RPC], F8)
            an = [
                [wpool.tile([128, 2, 512], F8, name=f"an{h}_{c}") for c in range(NCH)]
                for h in range(2)
            ]
            # hB (h=1) chunks + qnT are t0-critical: front of each queue
            nc.sync.dma_start(out=qnT[:], in_=qnT_d[:])
            nc.sync.dma_start(out=an[1][0][:], in_=anT_d[1, 0])
            nc.gpsimd.dma_start(out=an[1][1][:], in_=anT_d[1, 1])
            nc.gpsimd.dma_start(out=an[0][0][:], in_=anT_d[0, 0])
            nc.scalar.dma_start(out=an[1][2][:], in_=anT_d[1, 2])
            nc.scalar.dma_start(out=an[0][1][:], in_=anT_d[0, 1])
            nc.sync.dma_start(out=an[0][2][:], in_=anT_d[0, 2])

            acc = wpool.tile([128, IT, 2], F32)
            colacc = wpool.tile([128, RPC], F32)
            scrap = wpool.tile([128, HALFW], BF16)
            bias = wpool.tile([128, 1], F32)
            warm = wpool.tile([128, 2, 128], F8)
            nc.vector.memset(bias[:], -1.0 / TEMP)
            nc.vector.memset(colacc[:], 0.0)
            nc.vector.memset(warm[:], 0.0)

            # two garbage DoubleRow MMs to take PE out of its cold state
            psw = wppool.tile([128, 128], F32)
            with tc.high_priority():
                for _ in range(2):
                    nc.tensor.matmul(
                        psw[:],
                        warm[:],
                        warm[:],
                        start=True,
                        stop=True,
                        perf_mode=mybir.MatmulPerfMode.DoubleRow,
                    )

            for t in range(IT):
                lhsT = qnT[:, :, t * 128:(t + 1) * 128]

                # ---- hB half: plain ACT; DVE does rowsum + colacc add
                psB = ppool.tile([128, HALFW], F32, tag="ps")
                for c in range(NCH):
                    nc.tensor.matmul(
                        psB[:, c * 512:(c + 1) * 512],
                        lhsT,
                        an[1][c][:],
                        start=True,
                        stop=True,
                        perf_mode=mybir.MatmulPerfMode.DoubleRow,
                    )
                scB = spool.tile([128, HALFW], BF16)
                nc.scalar.activation(
                    scB[:],
                    psB[:],
                    mybir.ActivationFunctionType.Exp,
                    bias=bias[:],
                    scale=ASCALE,
                )
                nc.vector.tensor_reduce(
                    acc[:, t, 0:1],
                    scB[:],
                    axis=mybir.AxisListType.X,
                    op=mybir.AluOpType.add,
                )
                nc.vector.scalar_tensor_tensor(
                    colacc[:],
                    scB[:, 512:HALFW],
                    1.0,
                    colacc[:],
                    op0=mybir.AluOpType.bypass,
                    op1=mybir.AluOpType.add,
                )

                # ---- hA half: ACT accumulator does the row sum
                psA = ppool.tile([128, HALFW], F32, tag="ps")
                for c in range(NCH):
                    nc.tensor.matmul(
                        psA[:, c * 512:(c + 1) * 512],
                        lhsT,
                        an[0][c][:],
                        start=True,
                        stop=True,
                        perf_mode=mybir.MatmulPerfMode.DoubleRow,
                    )
                nc.scalar.activation(
                    scrap[:],
                    psA[:],
                    mybir.ActivationFunctionType.Exp,
                    bias=bias[:],
                    scale=ASCALE,
                    accum_out=acc[:, t, 1:2],
                )

            nc.sync.dma_start(out=acc_d[:], in_=acc[:])
            nc.gpsimd.dma_start(out=col_d[:], in_=colacc[:])

    nc.compile()
    _CACHE["nc"] = nc
    return nc


def _prep_inputs(z_i, z_j):
    f8 = ml_dtypes.float8_e4m3
    zin = z_i / np.sqrt(np.sum(z_i * z_i, axis=1, keepdims=True))
    zjn = z_j / np.sqrt(np.sum(z_j * z_j, axis=1, keepdims=True))
    posn = np.sum(zin * zjn, axis=1, dtype=np.float64) / TEMP      # [4096]

    q8 = [(SC * zjn).astype(f8), (SC * zin).astype(f8)]
    # exact squared norms of the quantized rows: the device Gram diagonal
    dsq = [np.sum(b.astype(np.float64) ** 2, axis=1) for b in q8]

    in_maps = []
    for c in range(NCORES):
        v, s = divmod(c, NCORES // 2)
        b = q8[v]
        brot = np.roll(b, -s * RPC, axis=0)
        # column order: [own block | +2 block | +1 block]; +1 sits in
        # hB at local cols 512:1536 so ONE colacc slice covers it
        cols = np.concatenate(
            [brot[0:RPC], brot[2 * RPC:3 * RPC], brot[RPC:2 * RPC]], axis=0
        )                                               # [3072, 256]
        anT = np.ascontiguousarray(
            cols.T.reshape(2, 128, 2, NCH, 512).transpose(2, 3, 1, 0, 4)
        )
        slab = b[s * RPC:(s + 1) * RPC]
        qnT = np.ascontiguousarray(slab.T.reshape(2, 128, RPC).transpose(1, 0, 2))
        in_maps.append({"anT": anT, "qnT": qnT})
    return in_maps, posn, dsq


def kernel(z_i, z_j):
    z_i = np.asarray(z_i, dtype=np.float32)
    z_j = np.asarray(z_j, dtype=np.float32)

    from concourse.bass_utils import run_bass_kernel_spmd

    nc = _build_program()
    in_maps, posn, dsq = _prep_inputs(z_i, z_j)

    res = run_bass_kernel_spmd(nc, in_maps, list(range(NCORES)))
    _CACHE["last_results"] = res

    nv = NCORES // 2
    rowsum = np.empty(2 * N, dtype=np.float64)
    colsum = np.empty((2, nv, RPC), dtype=np.float64)
    for c in range(NCORES):
        v, s = divmod(c, nv)
        a = res.results[c]["acc"].astype(np.float64)   # [128, IT, 2]
        rowsum[c * RPC:(c + 1) * RPC] = a.sum(axis=2).T.reshape(-1)
        colsum[v, s] = res.results[c]["colacc"].astype(np.float64).sum(axis=0)
    for v in range(2):
        for s in range(nv):
            # slab s's missing (s, s+3) block rowsums = colsums of the
            # +1 block computed by core (v, s-1)
            g0 = v * N + s * RPC
            rowsum[g0:g0 + RPC] += colsum[v, (s - 1) % nv]

    dsq_g = np.concatenate(dsq)                        # [8192] |q8 row|^2
    rowsum -= np.exp(dsq_g * ASCALE - 1.0 / TEMP)      # exact diagonal removal

    posn_g = np.concatenate([posn, posn])
    epos_g = np.exp(posn_g - 1.0 / TEMP)

    lse = 1.0 / TEMP + np.log(rowsum + epos_g)
    loss = np.mean(lse - posn_g)
    return np.array(loss, dtype=np.float32)


# revision 24
# speedup vs baseline: 1.4706x; 1.0253x over previous
"""NT-Xent loss on 8 Trainium2 cores (v4: cyclic 3-block symmetry, 75% exp work).

Math: with row-normalized views zjn, zin and r = [zjn; zin],
S = r@r.T / T, pos_i = (zjn_i . zin_i)/T, the kept logits for row i are
the same-view off-diagonal entries plus pos_i.  All cosine logits are
<= 1/T = 10, so with the fixed shift 10:

  lse_i  = 10 + ln( rowsum_i + epos_i )
  loss   = mean(lse_i - pos_i)

where rowsum_i = sum_{j != i} exp(S_same[i,j] - 10) and
epos_i = exp(pos_i - 10).

Symmetry: each view's 4096x4096 Gram is split into 4x4 blocks of
1024.  Core (v,s) computes its row-slab against column blocks
{s, s+1, s+2} (cyclic), i.e. 3072 of 4096 columns.  The missing block
(s, s+3) equals block (s+3, s).T, which core (v, s+3) computes as its
"+1" block; its COLUMN sums are that block's row sums.  So each core
also accumulates the column sums of its +1 block (DVE adds into a
[128,1024] fp32 accumulator; host finishes the partition reduction).

Device per tile t (hB first so the accum-ACT ends the chain):
  3 DoubleRow fp8 MMs -> psB[128,1536]; ACT exp -> scB;
  DVE row-reduce scB -> acc[:,t,0]; DVE colacc += scB[:,512:1536];
  3 MMs -> psA; ACT exp with accum_out=acc[:,t,1] (out to scrap).
Two garbage DoubleRow warm-up MMs run first to take the PE out of its
cold state.  Host: rowsum(v,s) = slabsum + colsum from core (v,s-1)
- exact fp8 diagonal; then the O(N*D) rest (normalize, pos, log, mean).
"""

import numpy as np
import ml_dtypes

N = 4096
D = 256
TEMP = 0.1
NCORES = 8
RPC = 2 * N // NCORES          # 1024 rows per core
IT = RPC // 128                # 8 i-tiles of 128 rows
W = 3 * RPC                    # 3072 columns per core (3 cyclic blocks)
HALFW = W // 2                 # 1536 cols per PSUM buffer / ACT op
NCH = HALFW // 512             # 3 column chunks per half
SC = 16.0                      # fp8 prescale (power of 2, exact)
ASCALE = (1.0 / TEMP) / (SC * SC)   # 10/256 applied in ACT

_CACHE = {}


def _build_program():
    if "nc" in _CACHE:
        return _CACHE["nc"]

    import concourse.bass as bass
    import concourse.tile as tile
    from concourse import bacc, mybir

    F8 = mybir.dt.float8e4
    BF16 = mybir.dt.bfloat16
    F32 = mybir.dt.float32

    nc = bacc.Bacc(
        "TRN2", target_bir_lowering=False, debug=False, num_devices=NCORES
    )

    # anT[h][c][p][k][col] = cols[h*1536 + c*512 + col, k*128 + p]
    anT_d = nc.dram_tensor("anT", [2, NCH, 128, 2, 512], F8, kind="ExternalInput")
    # qnT[p][k][r] = q8slab[r, k*128 + p]
    qnT_d = nc.dram_tensor("qnT", [128, 2, RPC], F8, kind="ExternalInput")
    acc_d = nc.dram_tensor("acc", [128, IT, 2], F32, kind="ExternalOutput")
    col_d = nc.dram_tensor("colacc", [128, RPC], F32, kind="ExternalOutput")

    with tile.TileContext(nc) as tc:
        with (
            tc.tile_pool(name="weights", bufs=1) as wpool,
            tc.tile_pool(name="scratch", bufs=2) as spool,
            tc.tile_pool(name="psum", bufs=2, space="PSUM") as ppool,
            tc.tile_pool(name="pwarm", bufs=1, space="PSUM") as wppool,
        ):
            qnT = wpool.tile([128, 2, RPC], F8)
            an = [
                [wpool.tile([128, 2, 512], F8, name=f"an{h}_{c}") for c in range(NCH)]
                for h in range(2)
            ]
            # DMA transfers serialize per queue: qnT and the first hB
            # chunk (the two t0-MM gates) must each be FIRST on a queue
            nc.sync.dma_start(out=qnT[:], in_=qnT_d[:])
            nc.gpsimd.dma_start(out=an[1][0][:], in_=anT_d[1, 0])
            nc.scalar.dma_start(out=an[1][1][:], in_=anT_d[1, 1])
            nc.scalar.dma_start(out=an[1][2][:], in_=anT_d[1, 2])
            nc.gpsimd.dma_start(out=an[0][0][:], in_=anT_d[0, 0])
            nc.sync.dma_start(out=an[0][1][:], in_=anT_d[0, 1])
            nc.scalar.dma_start(out=an[0][2][:], in_=anT_d[0, 2])

            acc = wpool.tile([128, IT, 2], F32)
            colacc = wpool.tile([128, RPC], F32)
            scrap = wpool.tile([128, HALFW], BF16)
            bias = wpool.tile([128, 1], F32)
            warm = wpool.tile([128, 2, 128], F8)
            nc.vector.memset(bias[:], -1.0 / TEMP)
            nc.vector.memset(colacc[:], 0.0)
            nc.vector.memset(warm[:], 0.0)

            # two garbage DoubleRow MMs to take PE out of its cold state
            psw = wppool.tile([128, 128], F32)
            with tc.high_priority():
                for _ in range(2):
                    nc.tensor.matmul(
                        psw[:],
                        warm[:],
                        warm[:],
                        start=True,
                        stop=True,
                        perf_mode=mybir.MatmulPerfMode.DoubleRow,
                    )

            for t in range(IT):
                lhsT = qnT[:, :, t * 128:(t + 1) * 128]

                # ---- hB half: plain ACT; DVE does rowsum + colacc add
                psB = ppool.tile([128, HALFW], F32, tag="ps")
                for c in range(NCH):
                    nc.tensor.matmul(
                        psB[:, c * 512:(c + 1) * 512],
                        lhsT,
                        an[1][c][:],
                        start=True,
                        stop=True,
                        perf_mode=mybir.MatmulPerfMode.DoubleRow,
                    )
                scB = spool.tile([128, HALFW], BF16)
                last = t == IT - 1
                # last tile: ACT accumulator does the hB rowsum so the
                # final colacc stt (and its output DMA) isn't stuck
                # behind a trailing DVE reduce
                nc.scalar.activation(
                    scB[:],
                    psB[:],
                    mybir.ActivationFunctionType.Exp,
                    bias=bias[:],
                    scale=ASCALE,
                    accum_out=acc[:, t, 0:1] if last else None,
                )
                nc.vector.scalar_tensor_tensor(
                    colacc[:],
                    scB[:, 512:HALFW],
                    1.0,
                    colacc[:],
                    op0=mybir.AluOpType.bypass,
                    op1=mybir.AluOpType.add,
                )
                if not last:
                    nc.vector.tensor_reduce(
                        acc[:, t, 0:1],
                        scB[:],
                        axis=mybir.AxisListType.X,
                        op=mybir.AluOpType.add,
                    )

                # ---- hA half: ACT accumulator does the row sum
                psA = ppool.tile([128, HALFW], F32, tag="ps")
                for c in range(NCH):
                    nc.tensor.matmul(
                        psA[:, c * 512:(c + 1) * 512],
                        lhsT,
                        an[0][c][:],
                        start=True,
                        stop=True,
                        perf_mode=mybir.MatmulPerfMode.DoubleRow,
                    )
                nc.scalar.activation(
                    scrap[:],
                    psA[:],
                    mybir.ActivationFunctionType.Exp,
                    bias=bias[:],
                    scale=ASCALE,
                    accum_out=acc[:, t, 1:2],
                )

            # tiles 0..6 of acc are final once t6's RA lands; only the
            # last slice waits for the end of the ACT chain
            nc.sync.dma_start(out=acc_d[:, 0:IT - 1], in_=acc[:, 0:IT - 1])
            nc.gpsimd.dma_start(out=col_d[:], in_=colacc[:])
            nc.sync.dma_start(out=acc_d[:, IT - 1:IT], in_=acc[:, IT - 1:IT])

    nc.compile()
    _CACHE["nc"] = nc
    return nc


def _prep_inputs(z_i, z_j):
    f8 = ml_dtypes.float8_e4m3
    zin = z_i / np.sqrt(np.sum(z_i * z_i, axis=1, keepdims=True))
    zjn = z_j / np.sqrt(np.sum(z_j * z_j, axis=1, keepdims=True))
    posn = np.sum(zin * zjn, axis=1, dtype=np.float64) / TEMP      # [4096]

    q8 = [(SC * zjn).astype(f8), (SC * zin).astype(f8)]
    # exact squared norms of the quantized rows: the device Gram diagonal
    dsq = [np.sum(b.astype(np.float64) ** 2, axis=1) for b in q8]

    in_maps = []
    for c in range(NCORES):
        v, s = divmod(c, NCORES // 2)
        b = q8[v]
        brot = np.roll(b, -s * RPC, axis=0)
        # column order: [own block | +2 block | +1 block]; +1 sits in
        # hB at local cols 512:1536 so ONE colacc slice covers it
        cols = np.concatenate(
            [brot[0:RPC], brot[2 * RPC:3 * RPC], brot[RPC:2 * RPC]], axis=0
        )                                               # [3072, 256]
        anT = np.ascontiguousarray(
            cols.T.reshape(2, 128, 2, NCH, 512).transpose(2, 3, 1, 0, 4)
        )
        slab = b[s * RPC:(s + 1) * RPC]
        qnT = np.ascontiguousarray(slab.T.reshape(2, 128, RPC).transpose(1, 0, 2))
        in_maps.append({"anT": anT, "qnT": qnT})
    return in_maps, posn, dsq


def kernel(z_i, z_j):
    z_i = np.asarray(z_i, dtype=np.float32)
    z_j = np.asarray(z_j, dtype=np.float32)

    from concourse.bass_utils import run_bass_kernel_spmd

    nc = _build_program()
    in_maps, posn, dsq = _prep_inputs(z_i, z_j)

    res = run_bass_kernel_spmd(nc, in_maps, list(range(NCORES)))
    _CACHE["last_results"] = res

    nv = NCORES // 2
    rowsum = np.empty(2 * N, dtype=np.float64)
    colsum = np.empty((2, nv, RPC), dtype=np.float64)
    for c in range(NCORES):
        v, s = divmod(c, nv)
        a = res.results[c]["acc"].astype(np.float64)   # [128, IT, 2]
        rowsum[c * RPC:(c + 1) * RPC] = a.sum(axis=2).T.reshape(-1)
        colsum[v, s] = res.results[c]["colacc"].astype(np.float64).sum(axis=0)
    for v in range(2):
        for s in range(nv):
            # slab s's missing (s, s+3) block rowsums = colsums of the
            # +1 block computed by core (v, s-1)
            g0 = v * N + s * RPC
            rowsum[g0:g0 + RPC] += colsum[v, (s - 1) % nv]

    dsq_g = np.concatenate(dsq)                        # [8192] |q8 row|^2
    rowsum -= np.exp(dsq_g * ASCALE - 1.0 / TEMP)      # exact diagonal removal

    posn_g = np.concatenate([posn, posn])
    epos_g = np.exp(posn_g - 1.0 / TEMP)

    lse = 1.0 / TEMP + np.log(rowsum + epos_g)
    loss = np.mean(lse - posn_g)
    return np.array(loss, dtype=np.float32)


# revision 25
# speedup vs baseline: 1.5036x; 1.0224x over previous
"""NT-Xent loss on 8 Trainium2 cores (v4: cyclic 3-block symmetry, 75% exp work).

Math: with row-normalized views zjn, zin and r = [zjn; zin],
S = r@r.T / T, pos_i = (zjn_i . zin_i)/T, the kept logits for row i are
the same-view off-diagonal entries plus pos_i.  All cosine logits are
<= 1/T = 10, so with the fixed shift 10:

  lse_i  = 10 + ln( rowsum_i + epos_i )
  loss   = mean(lse_i - pos_i)

where rowsum_i = sum_{j != i} exp(S_same[i,j] - 10) and
epos_i = exp(pos_i - 10).

Symmetry: each view's 4096x4096 Gram is split into 4x4 blocks of
1024.  Core (v,s) computes its row-slab against column blocks
{s, s+1, s+2} (cyclic), i.e. 3072 of 4096 columns.  The missing block
(s, s+3) equals block (s+3, s).T, which core (v, s+3) computes as its
"+1" block; its COLUMN sums are that block's row sums.  So each core
also accumulates the column sums of its +1 block (DVE adds into a
[128,1024] fp32 accumulator; host finishes the partition reduction).

Device per tile t (hB first so the accum-ACT ends the chain):
  3 DoubleRow fp8 MMs -> psB[128,1536]; ACT exp -> scB;
  DVE row-reduce scB -> acc[:,t,0]; DVE colacc += scB[:,512:1536];
  3 MMs -> psA; ACT exp with accum_out=acc[:,t,1] (out to scrap).
Two garbage DoubleRow warm-up MMs run first to take the PE out of its
cold state.  Host: rowsum(v,s) = slabsum + colsum from core (v,s-1)
- exact fp8 diagonal; then the O(N*D) rest (normalize, pos, log, mean).
"""

import numpy as np
import ml_dtypes

N = 4096
D = 256
TEMP = 0.1
NCORES = 8
RPC = 2 * N // NCORES          # 1024 rows per core
IT = RPC // 128                # 8 i-tiles of 128 rows
W = 3 * RPC                    # 3072 columns per core (3 cyclic blocks)
HALFW = W // 2                 # 1536 cols per PSUM buffer / ACT op
NCH = HALFW // 512             # 3 column chunks per half
SC = 16.0                      # fp8 prescale (power of 2, exact)
ASCALE = (1.0 / TEMP) / (SC * SC)   # 10/256 applied in ACT

_CACHE = {}


def _build_program():
    if "nc" in _CACHE:
        return _CACHE["nc"]

    import concourse.bass as bass
    import concourse.tile as tile
    from concourse import bacc, mybir

    F8 = mybir.dt.float8e4
    BF16 = mybir.dt.bfloat16
    F32 = mybir.dt.float32

    nc = bacc.Bacc(
        "TRN2", target_bir_lowering=False, debug=False, num_devices=NCORES
    )

    # anT[h][c][p][k][col] = cols[h*1536 + c*512 + col, k*128 + p]
    anT_d = nc.dram_tensor("anT", [2, NCH, 128, 2, 512], F8, kind="ExternalInput")
    # qnT[p][k][r] = q8slab[r, k*128 + p]
    qnT_d = nc.dram_tensor("qnT", [128, 2, RPC], F8, kind="ExternalInput")
    acc_d = nc.dram_tensor("acc", [128, IT, 2], F32, kind="ExternalOutput")
    col_d = nc.dram_tensor("colacc", [128, RPC], F32, kind="ExternalOutput")

    with tile.TileContext(nc) as tc:
        with (
            tc.tile_pool(name="weights", bufs=1) as wpool,
            tc.tile_pool(name="scratch", bufs=2) as spool,
            tc.tile_pool(name="psum", bufs=2, space="PSUM") as ppool,
            tc.tile_pool(name="pwarm", bufs=1, space="PSUM") as wppool,
        ):
            qnT = wpool.tile([128, 2, RPC], F8)
            an = [
                [wpool.tile([128, 2, 512], F8, name=f"an{h}_{c}") for c in range(NCH)]
                for h in range(2)
            ]
            # DMA transfers serialize per queue, and gpsimd issues its
            # first DMA ~0.7us later than sync/scalar.  The two t0-MM
            # gates (qnT's first 128 cols, an[1][0]) go FIRST on the two
            # early queues; qnT is split so t0 waits on 32KB, not 256KB.
            nc.sync.dma_start(out=qnT[:, :, 0:128], in_=qnT_d[:, :, 0:128])
            nc.scalar.dma_start(out=an[1][0][:], in_=anT_d[1, 0])
            nc.gpsimd.dma_start(out=an[1][1][:], in_=anT_d[1, 1])
            nc.sync.dma_start(out=qnT[:, :, 128:RPC], in_=qnT_d[:, :, 128:RPC])
            nc.scalar.dma_start(out=an[1][2][:], in_=anT_d[1, 2])
            nc.gpsimd.dma_start(out=an[0][0][:], in_=anT_d[0, 0])
            nc.sync.dma_start(out=an[0][1][:], in_=anT_d[0, 1])
            nc.scalar.dma_start(out=an[0][2][:], in_=anT_d[0, 2])

            acc = wpool.tile([128, IT, 2], F32)
            colacc = wpool.tile([128, RPC], F32)
            scrap = wpool.tile([128, HALFW], BF16)
            bias = wpool.tile([128, 1], F32)
            warm = wpool.tile([128, 2, 128], F8)
            nc.vector.memset(bias[:], -1.0 / TEMP)
            nc.vector.memset(colacc[:], 0.0)
            nc.vector.memset(warm[:], 0.0)

            # two garbage DoubleRow MMs to take PE out of its cold state
            psw = wppool.tile([128, 128], F32)
            with tc.high_priority():
                for _ in range(2):
                    nc.tensor.matmul(
                        psw[:],
                        warm[:],
                        warm[:],
                        start=True,
                        stop=True,
                        perf_mode=mybir.MatmulPerfMode.DoubleRow,
                    )

            for t in range(IT):
                lhsT = qnT[:, :, t * 128:(t + 1) * 128]

                # ---- hB half: plain ACT; DVE does rowsum + colacc add
                psB = ppool.tile([128, HALFW], F32, tag="ps")
                for c in range(NCH):
                    nc.tensor.matmul(
                        psB[:, c * 512:(c + 1) * 512],
                        lhsT,
                        an[1][c][:],
                        start=True,
                        stop=True,
                        perf_mode=mybir.MatmulPerfMode.DoubleRow,
                    )
                scB = spool.tile([128, HALFW], BF16)
                last = t == IT - 1
                # last tile: ACT accumulator does the hB rowsum so the
                # final colacc stt (and its output DMA) isn't stuck
                # behind a trailing DVE reduce
                nc.scalar.activation(
                    scB[:],
                    psB[:],
                    mybir.ActivationFunctionType.Exp,
                    bias=bias[:],
                    scale=ASCALE,
                    accum_out=acc[:, t, 0:1] if last else None,
                )
                nc.vector.scalar_tensor_tensor(
                    colacc[:],
                    scB[:, 512:HALFW],
                    1.0,
                    colacc[:],
                    op0=mybir.AluOpType.bypass,
                    op1=mybir.AluOpType.add,
                )
                if not last:
                    nc.vector.tensor_reduce(
                        acc[:, t, 0:1],
                        scB[:],
                        axis=mybir.AxisListType.X,
                        op=mybir.AluOpType.add,
                    )

                # ---- hA half: ACT accumulator does the row sum
                psA = ppool.tile([128, HALFW], F32, tag="ps")
                for c in range(NCH):
                    nc.tensor.matmul(
                        psA[:, c * 512:(c + 1) * 512],
                        lhsT,
                        an[0][c][:],
                        start=True,
                        stop=True,
                        perf_mode=mybir.MatmulPerfMode.DoubleRow,
                    )
                nc.scalar.activation(
                    scrap[:],
                    psA[:],
                    mybir.ActivationFunctionType.Exp,
                    bias=bias[:],
                    scale=ASCALE,
                    accum_out=acc[:, t, 1:2],
                )

            # tiles 0..6 of acc are final once t6's RA lands; only the
            # last slice waits for the end of the ACT chain
            nc.sync.dma_start(out=acc_d[:, 0:IT - 1], in_=acc[:, 0:IT - 1])
            nc.gpsimd.dma_start(out=col_d[:], in_=colacc[:])
            nc.sync.dma_start(out=acc_d[:, IT - 1:IT], in_=acc[:, IT - 1:IT])

    nc.compile()
    _CACHE["nc"] = nc
    return nc


def _prep_inputs(z_i, z_j):
    f8 = ml_dtypes.float8_e4m3
    zin = z_i / np.sqrt(np.sum(z_i * z_i, axis=1, keepdims=True))
    zjn = z_j / np.sqrt(np.sum(z_j * z_j, axis=1, keepdims=True))
    posn = np.sum(zin * zjn, axis=1, dtype=np.float64) / TEMP      # [4096]

    q8 = [(SC * zjn).astype(f8), (SC * zin).astype(f8)]
    # exact squared norms of the quantized rows: the device Gram diagonal
    dsq = [np.sum(b.astype(np.float64) ** 2, axis=1) for b in q8]

    in_maps = []
    for c in range(NCORES):
        v, s = divmod(c, NCORES // 2)
        b = q8[v]
        brot = np.roll(b, -s * RPC, axis=0)
        # column order: [own block | +2 block | +1 block]; +1 sits in
        # hB at local cols 512:1536 so ONE colacc slice covers it
        cols = np.concatenate(
            [brot[0:RPC], brot[2 * RPC:3 * RPC], brot[RPC:2 * RPC]], axis=0
        )                                               # [3072, 256]
        anT = np.ascontiguousarray(
            cols.T.reshape(2, 128, 2, NCH, 512).transpose(2, 3, 1, 0, 4)
        )
        slab = b[s * RPC:(s + 1) * RPC]
        qnT = np.ascontiguousarray(slab.T.reshape(2, 128, RPC).transpose(1, 0, 2))
        in_maps.append({"anT": anT, "qnT": qnT})
    return in_maps, posn, dsq


def kernel(z_i, z_j):
    z_i = np.asarray(z_i, dtype=np.float32)
    z_j = np.asarray(z_j, dtype=np.float32)

    from concourse.bass_utils import run_bass_kernel_spmd

    nc = _build_program()
    in_maps, posn, dsq = _prep_inputs(z_i, z_j)

    res = run_bass_kernel_spmd(nc, in_maps, list(range(NCORES)))
    _CACHE["last_results"] = res

    nv = NCORES // 2
    rowsum = np.empty(2 * N, dtype=np.float64)
    colsum = np.empty((2, nv, RPC), dtype=np.float64)
    for c in range(NCORES):
        v, s = divmod(c, nv)
        a = res.results[c]["acc"].astype(np.float64)   # [128, IT, 2]
        rowsum[c * RPC:(c + 1) * RPC] = a.sum(axis=2).T.reshape(-1)
        colsum[v, s] = res.results[c]["colacc"].astype(np.float64).sum(axis=0)
    for v in range(2):
        for s in range(nv):
            # slab s's missing (s, s+3) block rowsums = colsums of the
            # +1 block computed by core (v, s-1)
            g0 = v * N + s * RPC
            rowsum[g0:g0 + RPC] += colsum[v, (s - 1) % nv]

    dsq_g = np.concatenate(dsq)                        # [8192] |q8 row|^2
    rowsum -= np.exp(dsq_g * ASCALE - 1.0 / TEMP)      # exact diagonal removal

    posn_g = np.concatenate([posn, posn])
    epos_g = np.exp(posn_g - 1.0 / TEMP)

    lse = 1.0 / TEMP + np.log(rowsum + epos_g)
    loss = np.mean(lse - posn_g)
    return np.array(loss, dtype=np.float32)
